# revision 16
# baseline (speedup 1.0000x reference)
"""Trainium2 Bass kernel for nn_MoE_29927332118881.

Math (reference): for each of N=48 rows (B=16 x CH=3) and K=4 anisotropic
2-D Gaussians over a 384x384 unit grid:
    e_k(h,w) = exp(-0.5 * || Sigma_k^T d ||^2),  d = (x(h)-mux_k, y(w)-muy_k)
    out = clip( sum_k w_k e_k / max(eps, sum_k e_k), 0, 1 )

Strategy (validated numerically on host, absmax err ~4e-3 vs 2e-2 tol):
the Gaussians are very smooth on this grid, so evaluate exp only on a
coarse h-grid (8 samples per 128-row output chunk). ScalarE computes
e = exp(scale_p * in + bias_p) with the separable-in-w factor folded
into the input tile (in = y + qc/scale, with a |scale|<tau fallback),
one [128,384] instruction per group of 4 chunks. One f16 matmul per
(group, num|den) contracts k and cubic-interpolates to a 16-row mid
grid packed 8 chunks per [128,384] PSUM pair; VectorE takes the ratio
y = clip(num*recip_fast(den),0,1) there; TensorE cubic-interpolates
mid->fine (one K=32 matmul per chunk into 2-bank pair tiles);
ScalarE/VectorE evict PSUM->SBUF as f16 (the only PSUM-capable
engines), two chunks per instruction, and each pair DMAs out in one
trigger. Output is f16; the host upcasts (quantization ~5e-4).

Layout: chunk t (n = t//3 of the 6 local rows, c = t%3 of 3 h-chunks)
owns partitions 32q:32q+32 = (k=4 x j=8 coarse rows) of group g = t//4
(q = t%4). Supers of 2 groups (8 chunks) share one [128,384] num/den
PSUM pair. 18 real chunks are padded with 2 dummy chunks (e=1,
num-weight 0) so every PSUM row read is written.

Sharding: data-parallel over the params batch dim: core i gets 6 of the
48 (B,CH) rows. The grid is replicated.
"""

import numpy as np

import concourse.bass as bass
import concourse.bacc as bacc
import concourse.mybir as mybir
from concourse.tile import TileContext
from concourse.bass_utils import run_bass_kernel_spmd

import concourse.dve_ops as dve_ops_mod
from concourse.dve_spec import Spec, Src0, Src1, C0, relu, minn, lower, _has_src1
from concourse.dve_uop import DveOpSpec


def _ensure_clip_mul_op():
    """Fused custom-DVE op: out = min(relu(in0) * in1, s0)."""
    for op in dve_ops_mod.OPS:
        if op.name == "CLIP_MUL_ANT":
            return op
    spec = Spec(
        body=minn(relu(Src0) * Src1, C0),
        reference=lambda in0, in1, s0, s1, imm2: np.minimum(
            np.maximum(in0.astype(np.float32), 0.0) * in1, s0
        ).astype(np.float32),
    )
    row = max(dve_ops_mod._SUB_OPCODE_FOR_NAME.values()) + 1
    assert row < 0x20
    dve_ops_mod._SUB_OPCODE_FOR_NAME["CLIP_MUL_ANT"] = row
    shas = {}
    for ver in ("v3", "v4"):
        s = DveOpSpec(name="CLIP_MUL_ANT", opcode=row,
                      uops=lower(spec, ver=ver), rd1_en=_has_src1(spec))
        shas[ver] = s.sha(ver)
    op = dve_ops_mod.DveOp("CLIP_MUL_ANT", spec, False, shas)
    dve_ops_mod.OPS.append(op)
    dve_ops_mod.CUSTOM_DVE_SPECS["CLIP_MUL_ANT"] = spec
    return op


F32 = mybir.dt.float32
F16 = mybir.dt.float16

H = 384
W = 384
K = 4
N_CORES = 8
N_LOC = 6                  # (B*CH) rows per core
NCO = 8                    # coarse exp samples per 128-row chunk (per k)
NMID = 16                  # mid-grid ratio samples per chunk
NCHUNK = 18                # real chunks per core (6 rows x 3)
NPAD = 20                  # incl 2 dummy chunks -> 5 full groups
NGRP = 5                   # groups (4 chunks each)
SUPERS = ((0, (0, 1)), (1, (2, 3)), (2, (4,)))  # super -> groups
MARGIN_C = 3.0             # coarse grid margin (pixels)
MARGIN_M = 2.0             # mid grid margin
TAU = 1e-3                 # |scale| threshold for the qc-fold fallback
# chunk-pairs whose PSUM->SBUF eviction runs on ScalarE (rest VectorE)
ACT_EVICT_PAIRS = frozenset((0, 2, 4, 6, 8))

_cache = {}


def _build_nc():
    nc = bacc.Bacc(target_bir_lowering=False)

    # consts: [:, 2g] scale col; [:, 2g+1] bias col
    consts_d = nc.dram_tensor("consts", [128, 2 * NGRP], F32,
                              kind="ExternalInput")
    # per-group ACT input tile: y + qc/scale (or qc where |scale|<tau);
    # group-major so each per-group transfer is DRAM-contiguous
    ybcq_d = nc.dram_tensor("ybcq", [NGRP, 128, W], F32, kind="ExternalInput")
    statn_d = nc.dram_tensor("statn", [NGRP, 128, 4 * NMID], F16,
                             kind="ExternalInput")
    statd_d = nc.dram_tensor("statd", [128, 4 * NMID], F16, kind="ExternalInput")
    # mid->fine interp: 2 variants (chunk parity in 32-row strip), rows
    # replicated per 32-strip so any strip slice has the right content
    m2_d = nc.dram_tensor("m2pad", [128, 2, 128], F16, kind="ExternalInput")
    out_d = nc.dram_tensor("out", [128, NPAD, W], F16, kind="ExternalOutput")

    clip_op = _ensure_clip_mul_op()

    with TileContext(nc) as tc:
        with (
            tc.tile_pool(name="const", bufs=1) as constp,
            tc.tile_pool(name="e1p", bufs=NGRP) as e1p,
            tc.tile_pool(name="rp", bufs=3) as rp,
            tc.tile_pool(name="ymidp", bufs=3) as ymidp,
            tc.tile_pool(name="youtp", bufs=9) as youtp,
            tc.tile_pool(name="nump", bufs=2, space="PSUM") as nump,
            tc.tile_pool(name="denp", bufs=2, space="PSUM") as denp,
            tc.tile_pool(name="yfp", bufs=2, space="PSUM") as yfp,
        ):
            # dummy activation first: hoists the exp ACT_TABLE_LOAD to t=0
            # so it overlaps the input DMAs
            warm = constp.tile([1, 2], F32)
            nc.vector.memset(warm[:, 0:1], 0.0)
            nc.scalar.activation(warm[:, 1:2], warm[:, 0:1],
                                 mybir.ActivationFunctionType.Exp)

            consts = constp.tile([128, 2 * NGRP], F32)
            ybcq = constp.tile([128, NGRP, W], F32)
            statn = constp.tile([128, NGRP, 4 * NMID], F16)
            statd = constp.tile([128, 4 * NMID], F16)
            m2 = constp.tile([128, 2, 128], F16)
            # split input transfers across both trigger queues, group-0
            # data first so the first exp can start early
            nc.sync.dma_start(out=ybcq[:, 0, :], in_=ybcq_d[0])
            nc.gpsimd.dma_start(out=consts[:], in_=consts_d[:])
            nc.gpsimd.dma_start(out=ybcq[:, 1, :], in_=ybcq_d[1])
            nc.sync.dma_start(out=ybcq[:, 2, :], in_=ybcq_d[2])
            nc.gpsimd.dma_start(out=ybcq[:, 3, :], in_=ybcq_d[3])
            nc.sync.dma_start(out=ybcq[:, 4, :], in_=ybcq_d[4])
            nc.gpsimd.dma_start(out=statd[:], in_=statd_d[:])
            for g in range(NGRP):
                eng = nc.sync if g % 2 == 0 else nc.gpsimd
                eng.dma_start(out=statn[:, g, :], in_=statn_d[g])
            nc.gpsimd.dma_start(out=m2[:], in_=m2_d[:])

            e1_g = {}
            num_s = {}
            den_s = {}
            ymid_s = {}

            def do_exp(g):
                e1 = e1p.tile([128, W], F16, name=f"e1{g}", tag="e1")
                nc.scalar.activation(
                    e1[:], ybcq[:, g, :], mybir.ActivationFunctionType.Exp,
                    bias=consts[:, 2 * g + 1:2 * g + 2],
                    scale=consts[:, 2 * g:2 * g + 1],
                )
                e1_g[g] = e1

            def do_contract(s, groups):
                num_s[s] = nump.tile([128, W], F32, name=f"num{s}", tag="num")
                den_s[s] = denp.tile([128, W], F32, name=f"den{s}", tag="den")
                for gl, g in enumerate(groups):
                    rows = 64 * gl
                    nc.tensor.matmul(num_s[s][rows:rows + 64, :],
                                     statn[:, g, :], e1_g[g][:],
                                     start=True, stop=True)
                    nc.tensor.matmul(den_s[s][rows:rows + 64, :],
                                     statd[:], e1_g[g][:],
                                     start=True, stop=True)

            def do_finalize(s, nrows):
                r = rp.tile([128, W], F32, name=f"r{s}", tag="r")
                nc.vector.reciprocal_approx_fast(
                    out=r[0:nrows, :], in_=den_s[s][0:nrows, :])
                ymid = ymidp.tile([128, W], F16, name=f"ym{s}", tag="ym")
                nc.vector._custom_dve(
                    clip_op, out=ymid[0:nrows, :],
                    in0=num_s[s][0:nrows, :], in1=r[0:nrows, :], s0=1.0)
                ymid_s[s] = ymid

            def do_outputs(s):
                for t0 in range(8 * s, min(8 * s + 8, NCHUNK), 2):
                    yf = yfp.tile([128, 2, 512], F32, name=f"yf{t0}", tag="yf")
                    for i, t in enumerate((t0, t0 + 1)):
                        l = t - 8 * s          # chunk index within super
                        st = l // 2            # 32-row strip in ymid
                        q2 = l % 2             # position within strip
                        nc.tensor.matmul(
                            yf[:, i, 0:W],
                            m2[32 * st:32 * st + 32, q2, :],
                            ymid_s[s][32 * st:32 * st + 32, :],
                            start=True, stop=True,
                            tile_position=(32 * st, 0),
                        )
                    # PSUM->SBUF f16 eviction (only ScalarE/VectorE can),
                    # two chunks per instruction
                    yout = youtp.tile([128, 2, W], F16,
                                      name=f"yo{t0}", tag="yo")
                    if (t0 // 2) in ACT_EVICT_PAIRS:
                        nc.scalar.copy(yout[:], yf[:, :, 0:W])
                    else:
                        nc.vector.tensor_copy(out=yout[:], in_=yf[:, :, 0:W])
                    nc.sync.dma_start(
                        out=out_d[:, t0:t0 + 2, :],
                        in_=yout[:],
                    )

            # software-pipelined: contract super s+1 before finalizing s so
            # the in-order PE queue never stalls on the DVE finalize
            do_exp(0)
            do_exp(1)
            do_contract(0, SUPERS[0][1])
            do_exp(2)
            do_exp(3)
            do_contract(1, SUPERS[1][1])
            do_finalize(0, 128)
            do_outputs(0)
            do_exp(4)
            do_contract(2, SUPERS[2][1])
            do_finalize(1, 128)
            do_outputs(1)
            do_finalize(2, 64)
            do_outputs(2)
    nc.finalize()
    return nc


def _interp_weights(cp, fp, order=4):
    """Lagrange interpolation weights from points cp to points fp."""
    M = np.zeros((len(cp), len(fp)))
    for j, f in enumerate(fp):
        i = int(np.searchsorted(cp, f))
        lo = min(max(i - order // 2, 0), len(cp) - order)
        pts = cp[lo:lo + order]
        for a in range(order):
            L = 1.0
            for b in range(order):
                if a != b:
                    L *= (f - pts[b]) / (pts[a] - pts[b])
            M[lo + a, j] = L
    return M


def _host_precompute(params: np.ndarray):
    """Build per-core derived input arrays (float64 host math)."""
    P = np.asarray(params, dtype=np.float64).reshape(48, 28)
    mu_x = P[:, 0:4]
    mu_y = P[:, 4:8]
    wgt = P[:, 8:12]
    S00 = P[:, 12:28][:, 0::4]
    S10 = P[:, 12:28][:, 2::4]
    S11 = P[:, 12:28][:, 3::4]
    A = S00 ** 2
    Bq = 2.0 * S00 * S10
    C = S10 ** 2 + S11 ** 2

    yg = np.linspace(0.0, 1.0, W)

    # chunk-relative coarse/mid grids (identical geometry for every chunk)
    rel_c = np.linspace(-MARGIN_C, 127.0 + MARGIN_C, NCO)
    rel_m = np.linspace(-MARGIN_M, 127.0 + MARGIN_M, NMID)
    M1 = _interp_weights(rel_c, rel_m)                      # [NCO, NMID]
    M2 = _interp_weights(rel_m, np.arange(128.0))           # [NMID, 128]

    m2pad = np.zeros((128, 2, 128), dtype=np.float16)
    for st in range(4):
        for q2 in range(2):
            m2pad[32 * st + 16 * q2:32 * st + 16 * q2 + 16, q2, :] = \
                M2.astype(np.float16)

    # statd: same for every group; rows (q,k,j) -> cols (q,m)
    statd = np.zeros((128, 4 * NMID), dtype=np.float16)
    for q in range(4):
        for k in range(K):
            statd[32 * q + 8 * k:32 * q + 8 * k + 8,
                  NMID * q:NMID * (q + 1)] = M1.astype(np.float16)

    in_maps = []
    for core in range(N_CORES):
        consts = np.zeros((128, 2 * NGRP), dtype=np.float64)
        ybcq = np.zeros((NGRP, 128, W), dtype=np.float64)
        statn = np.zeros((NGRP, 128, 4 * NMID), dtype=np.float64)
        for t in range(NPAD):
            g, q = t // 4, t % 4
            rows0 = 32 * q
            if t >= NCHUNK:
                # dummy: e = exp(1*0 + 0) = 1; num weight 0, den from statd
                consts[rows0:rows0 + 32, 2 * g] = 1.0
                continue
            n = t // 3
            c0 = 128 * (t % 3)
            ng = core * N_LOC + n
            cp = (c0 + rel_c) / (H - 1)                     # unit coords
            for k in range(K):
                dxc = cp - mu_x[ng, k]
                qa = -0.5 * A[ng, k] * dxc * dxc
                dxB = -0.5 * Bq[ng, k] * dxc
                scale = dxB
                bias = qa - dxB * mu_y[ng, k]
                dy = yg - mu_y[ng, k]
                qc = -0.5 * C[ng, k] * dy * dy              # [W]
                for j in range(NCO):
                    row = rows0 + 8 * k + j
                    if abs(scale[j]) >= TAU:
                        consts[row, 2 * g] = scale[j]
                        consts[row, 2 * g + 1] = bias[j]
                        ybcq[g, row, :] = yg + qc / scale[j]
                    else:
                        # drop the (tiny) dxB*y term: |error| < tau
                        consts[row, 2 * g] = 1.0
                        consts[row, 2 * g + 1] = bias[j]
                        ybcq[g, row, :] = qc
                statn[g, rows0 + 8 * k:rows0 + 8 * k + 8,
                      NMID * q:NMID * (q + 1)] = M1 * wgt[ng, k]
            # f16 exp-overflow guard: shift this chunk's rows down
            # uniformly (cancels exactly in num/den); qc <= 0 so
            # bias + relu(scale) upper-bounds scale*y + qc + bias
            rows = slice(rows0, rows0 + 32)
            m = consts[rows, 2 * g + 1] + \
                np.maximum(consts[rows, 2 * g], 0.0)
            shift = max(0.0, m.max() - 9.0)
            if shift > 0.0:
                consts[rows, 2 * g + 1] -= shift
        in_maps.append({
            "consts": consts.astype(np.float32),
            "ybcq": ybcq.astype(np.float32),
            "statn": statn.astype(np.float32).astype(np.float16),
            "statd": statd,
            "m2pad": m2pad,
        })
    return in_maps


def _run(height, width, params, trace=False, **trace_kwargs):
    assert int(height) == H and int(width) == W, (height, width)
    if "nc" not in _cache:
        _cache["nc"] = _build_nc()
    nc = _cache["nc"]
    in_maps = _host_precompute(params)
    res = run_bass_kernel_spmd(
        nc, in_maps, core_ids=list(range(N_CORES)), trace=trace, **trace_kwargs
    )
    full = np.empty((48, H, W), dtype=np.float32)
    for core in range(N_CORES):
        o = res.results[core]["out"].astype(np.float32)  # [128, 20, 384] f16
        full[core * N_LOC:(core + 1) * N_LOC] = \
            o[:, :NCHUNK, :].transpose(1, 0, 2).reshape(N_LOC, H, W)
    return full.reshape(16, 3, H, W), res


def kernel(height, width, params):
    out, _ = _run(height, width, params)
    return out


# revision 17
# speedup vs baseline: 1.0092x; 1.0092x over previous
"""Trainium2 Bass kernel for nn_MoE_29927332118881.

Math (reference): for each of N=48 rows (B=16 x CH=3) and K=4 anisotropic
2-D Gaussians over a 384x384 unit grid:
    e_k(h,w) = exp(-0.5 * || Sigma_k^T d ||^2),  d = (x(h)-mux_k, y(w)-muy_k)
    out = clip( sum_k w_k e_k / max(eps, sum_k e_k), 0, 1 )

Strategy (validated numerically on host, absmax err ~4e-3 vs 2e-2 tol):
the Gaussians are very smooth on this grid, so evaluate exp only on a
coarse h-grid (8 samples per 128-row output chunk). ScalarE computes
e = exp(scale_p * in + bias_p) with the separable-in-w factor folded
into the input tile (in = y + qc/scale, with a |scale|<tau fallback),
one [128,384] instruction per group of 4 chunks. One f16 matmul per
(group, num|den) contracts k and cubic-interpolates to a 16-row mid
grid packed 8 chunks per [128,384] PSUM pair; VectorE takes the ratio
y = clip(num*recip_fast(den),0,1) there; TensorE cubic-interpolates
mid->fine (one K=32 matmul per chunk into 2-bank pair tiles);
ScalarE/VectorE evict PSUM->SBUF as f16 (the only PSUM-capable
engines), two chunks per instruction, and each pair DMAs out in one
trigger. Output is f16; the host upcasts (quantization ~5e-4).

Layout: chunk t (n = t//3 of the 6 local rows, c = t%3 of 3 h-chunks)
owns partitions 32q:32q+32 = (k=4 x j=8 coarse rows) of group g = t//4
(q = t%4). Supers of 2 groups (8 chunks) share one [128,384] num/den
PSUM pair. 18 real chunks are padded with 2 dummy chunks (e=1,
num-weight 0) so every PSUM row read is written.

Sharding: data-parallel over the params batch dim: core i gets 6 of the
48 (B,CH) rows. The grid is replicated.
"""

import numpy as np

import concourse.bass as bass
import concourse.bacc as bacc
import concourse.mybir as mybir
from concourse.tile import TileContext
from concourse.bass_utils import run_bass_kernel_spmd

import concourse.dve_ops as dve_ops_mod
from concourse.dve_spec import Spec, Src0, Src1, C0, relu, minn, lower, _has_src1
from concourse.dve_uop import DveOpSpec


def _ensure_clip_mul_op():
    """Fused custom-DVE op: out = min(relu(in0) * in1, s0)."""
    for op in dve_ops_mod.OPS:
        if op.name == "CLIP_MUL_ANT":
            return op
    spec = Spec(
        body=minn(relu(Src0) * Src1, C0),
        reference=lambda in0, in1, s0, s1, imm2: np.minimum(
            np.maximum(in0.astype(np.float32), 0.0) * in1, s0
        ).astype(np.float32),
    )
    row = max(dve_ops_mod._SUB_OPCODE_FOR_NAME.values()) + 1
    assert row < 0x20
    dve_ops_mod._SUB_OPCODE_FOR_NAME["CLIP_MUL_ANT"] = row
    shas = {}
    for ver in ("v3", "v4"):
        s = DveOpSpec(name="CLIP_MUL_ANT", opcode=row,
                      uops=lower(spec, ver=ver), rd1_en=_has_src1(spec))
        shas[ver] = s.sha(ver)
    op = dve_ops_mod.DveOp("CLIP_MUL_ANT", spec, False, shas)
    dve_ops_mod.OPS.append(op)
    dve_ops_mod.CUSTOM_DVE_SPECS["CLIP_MUL_ANT"] = spec
    return op


F32 = mybir.dt.float32
F16 = mybir.dt.float16

H = 384
W = 384
K = 4
N_CORES = 8
N_LOC = 6                  # (B*CH) rows per core
NCO = 8                    # coarse exp samples per 128-row chunk (per k)
NMID = 16                  # mid-grid ratio samples per chunk
NCHUNK = 18                # real chunks per core (6 rows x 3)
NPAD = 20                  # incl 2 dummy chunks -> 5 full groups
NGRP = 5                   # groups (4 chunks each)
SUPERS = ((0, (0, 1)), (1, (2, 3)), (2, (4,)))  # super -> groups
MARGIN_C = 3.0             # coarse grid margin (pixels)
MARGIN_M = 2.0             # mid grid margin
TAU = 1e-3                 # |scale| threshold for the qc-fold fallback
# chunk-pairs whose PSUM->SBUF eviction runs on ScalarE (rest VectorE)
ACT_EVICT_PAIRS = frozenset((0, 2, 3, 5, 6, 8))

_cache = {}


def _build_nc():
    nc = bacc.Bacc(target_bir_lowering=False)

    # consts: [:, 2g] scale col; [:, 2g+1] bias col
    consts_d = nc.dram_tensor("consts", [128, 2 * NGRP], F32,
                              kind="ExternalInput")
    # per-group ACT input tile: y + qc/scale (or qc where |scale|<tau);
    # group-major so each per-group transfer is DRAM-contiguous
    ybcq_d = nc.dram_tensor("ybcq", [NGRP, 128, W], F32, kind="ExternalInput")
    statn_d = nc.dram_tensor("statn", [NGRP, 128, 4 * NMID], F16,
                             kind="ExternalInput")
    statd_d = nc.dram_tensor("statd", [128, 4 * NMID], F16, kind="ExternalInput")
    # mid->fine interp: 2 variants (chunk parity in 32-row strip), rows
    # replicated per 32-strip so any strip slice has the right content
    m2_d = nc.dram_tensor("m2pad", [128, 2, 128], F16, kind="ExternalInput")
    out_d = nc.dram_tensor("out", [128, NPAD, W], F16, kind="ExternalOutput")

    clip_op = _ensure_clip_mul_op()

    with TileContext(nc) as tc:
        with (
            tc.tile_pool(name="const", bufs=1) as constp,
            tc.tile_pool(name="e1p", bufs=NGRP) as e1p,
            tc.tile_pool(name="rp", bufs=3) as rp,
            tc.tile_pool(name="ymidp", bufs=3) as ymidp,
            tc.tile_pool(name="youtp", bufs=9) as youtp,
            tc.tile_pool(name="nump", bufs=2, space="PSUM") as nump,
            tc.tile_pool(name="denp", bufs=2, space="PSUM") as denp,
            tc.tile_pool(name="yfp", bufs=2, space="PSUM") as yfp,
        ):
            # dummy activation first: hoists the exp ACT_TABLE_LOAD to t=0
            # so it overlaps the input DMAs
            warm = constp.tile([1, 2], F32)
            nc.vector.memset(warm[:, 0:1], 0.0)
            nc.scalar.activation(warm[:, 1:2], warm[:, 0:1],
                                 mybir.ActivationFunctionType.Exp)

            consts = constp.tile([128, 2 * NGRP], F32)
            ybcq = constp.tile([128, NGRP, W], F32)
            statn = constp.tile([128, NGRP, 4 * NMID], F16)
            statd = constp.tile([128, 4 * NMID], F16)
            m2 = constp.tile([128, 2, 128], F16)
            # split input transfers across both trigger queues, group-0
            # data first so the first exp can start early
            # group 0 split across both queues to halve first-exp latency
            nc.sync.dma_start(out=ybcq[0:64, 0, :], in_=ybcq_d[0, 0:64, :])
            nc.gpsimd.dma_start(out=ybcq[64:128, 0, :], in_=ybcq_d[0, 64:128, :])
            nc.gpsimd.dma_start(out=consts[:], in_=consts_d[:])
            nc.gpsimd.dma_start(out=ybcq[:, 1, :], in_=ybcq_d[1])
            nc.sync.dma_start(out=ybcq[:, 2, :], in_=ybcq_d[2])
            nc.gpsimd.dma_start(out=ybcq[:, 3, :], in_=ybcq_d[3])
            nc.sync.dma_start(out=ybcq[:, 4, :], in_=ybcq_d[4])
            nc.gpsimd.dma_start(out=statd[:], in_=statd_d[:])
            for g in range(NGRP):
                eng = nc.sync if g % 2 == 0 else nc.gpsimd
                eng.dma_start(out=statn[:, g, :], in_=statn_d[g])
            nc.gpsimd.dma_start(out=m2[:], in_=m2_d[:])

            e1_g = {}
            num_s = {}
            den_s = {}
            ymid_s = {}

            def do_exp(g):
                e1 = e1p.tile([128, W], F16, name=f"e1{g}", tag="e1")
                nc.scalar.activation(
                    e1[:], ybcq[:, g, :], mybir.ActivationFunctionType.Exp,
                    bias=consts[:, 2 * g + 1:2 * g + 2],
                    scale=consts[:, 2 * g:2 * g + 1],
                )
                e1_g[g] = e1

            def do_contract(s, groups):
                num_s[s] = nump.tile([128, W], F32, name=f"num{s}", tag="num")
                den_s[s] = denp.tile([128, W], F32, name=f"den{s}", tag="den")
                for gl, g in enumerate(groups):
                    rows = 64 * gl
                    nc.tensor.matmul(num_s[s][rows:rows + 64, :],
                                     statn[:, g, :], e1_g[g][:],
                                     start=True, stop=True)
                    nc.tensor.matmul(den_s[s][rows:rows + 64, :],
                                     statd[:], e1_g[g][:],
                                     start=True, stop=True)

            def do_finalize(s, nrows):
                r = rp.tile([128, W], F32, name=f"r{s}", tag="r")
                nc.vector.reciprocal_approx_fast(
                    out=r[0:nrows, :], in_=den_s[s][0:nrows, :])
                ymid = ymidp.tile([128, W], F16, name=f"ym{s}", tag="ym")
                nc.vector._custom_dve(
                    clip_op, out=ymid[0:nrows, :],
                    in0=num_s[s][0:nrows, :], in1=r[0:nrows, :], s0=1.0)
                ymid_s[s] = ymid

            def do_outputs(s):
                for t0 in range(8 * s, min(8 * s + 8, NCHUNK), 2):
                    yf = yfp.tile([128, 2, 512], F32, name=f"yf{t0}", tag="yf")
                    for i, t in enumerate((t0, t0 + 1)):
                        l = t - 8 * s          # chunk index within super
                        st = l // 2            # 32-row strip in ymid
                        q2 = l % 2             # position within strip
                        nc.tensor.matmul(
                            yf[:, i, 0:W],
                            m2[32 * st:32 * st + 32, q2, :],
                            ymid_s[s][32 * st:32 * st + 32, :],
                            start=True, stop=True,
                            tile_position=(32 * st, 0),
                        )
                    # PSUM->SBUF f16 eviction (only ScalarE/VectorE can),
                    # two chunks per instruction
                    yout = youtp.tile([128, 2, W], F16,
                                      name=f"yo{t0}", tag="yo")
                    if (t0 // 2) in ACT_EVICT_PAIRS:
                        nc.scalar.copy(yout[:], yf[:, :, 0:W])
                    else:
                        nc.vector.tensor_copy(out=yout[:], in_=yf[:, :, 0:W])
                    nc.sync.dma_start(
                        out=out_d[:, t0:t0 + 2, :],
                        in_=yout[:],
                    )

            # software-pipelined: contract super s+1 before finalizing s so
            # the in-order PE queue never stalls on the DVE finalize
            do_exp(0)
            do_exp(1)
            do_contract(0, SUPERS[0][1])
            do_exp(2)
            do_exp(3)
            do_contract(1, SUPERS[1][1])
            do_finalize(0, 128)
            do_outputs(0)
            do_exp(4)
            do_contract(2, SUPERS[2][1])
            do_finalize(1, 128)
            do_outputs(1)
            do_finalize(2, 64)
            do_outputs(2)
    nc.finalize()
    return nc


def _interp_weights(cp, fp, order=4):
    """Lagrange interpolation weights from points cp to points fp."""
    M = np.zeros((len(cp), len(fp)))
    for j, f in enumerate(fp):
        i = int(np.searchsorted(cp, f))
        lo = min(max(i - order // 2, 0), len(cp) - order)
        pts = cp[lo:lo + order]
        for a in range(order):
            L = 1.0
            for b in range(order):
                if a != b:
                    L *= (f - pts[b]) / (pts[a] - pts[b])
            M[lo + a, j] = L
    return M


def _host_precompute(params: np.ndarray):
    """Build per-core derived input arrays (float64 host math)."""
    P = np.asarray(params, dtype=np.float64).reshape(48, 28)
    mu_x = P[:, 0:4]
    mu_y = P[:, 4:8]
    wgt = P[:, 8:12]
    S00 = P[:, 12:28][:, 0::4]
    S10 = P[:, 12:28][:, 2::4]
    S11 = P[:, 12:28][:, 3::4]
    A = S00 ** 2
    Bq = 2.0 * S00 * S10
    C = S10 ** 2 + S11 ** 2

    yg = np.linspace(0.0, 1.0, W)

    # chunk-relative coarse/mid grids (identical geometry for every chunk)
    rel_c = np.linspace(-MARGIN_C, 127.0 + MARGIN_C, NCO)
    rel_m = np.linspace(-MARGIN_M, 127.0 + MARGIN_M, NMID)
    M1 = _interp_weights(rel_c, rel_m)                      # [NCO, NMID]
    M2 = _interp_weights(rel_m, np.arange(128.0))           # [NMID, 128]

    m2pad = np.zeros((128, 2, 128), dtype=np.float16)
    for st in range(4):
        for q2 in range(2):
            m2pad[32 * st + 16 * q2:32 * st + 16 * q2 + 16, q2, :] = \
                M2.astype(np.float16)

    # statd: same for every group; rows (q,k,j) -> cols (q,m)
    statd = np.zeros((128, 4 * NMID), dtype=np.float16)
    for q in range(4):
        for k in range(K):
            statd[32 * q + 8 * k:32 * q + 8 * k + 8,
                  NMID * q:NMID * (q + 1)] = M1.astype(np.float16)

    in_maps = []
    for core in range(N_CORES):
        consts = np.zeros((128, 2 * NGRP), dtype=np.float64)
        ybcq = np.zeros((NGRP, 128, W), dtype=np.float64)
        statn = np.zeros((NGRP, 128, 4 * NMID), dtype=np.float64)
        for t in range(NPAD):
            g, q = t // 4, t % 4
            rows0 = 32 * q
            if t >= NCHUNK:
                # dummy: e = exp(1*0 + 0) = 1; num weight 0, den from statd
                consts[rows0:rows0 + 32, 2 * g] = 1.0
                continue
            n = t // 3
            c0 = 128 * (t % 3)
            ng = core * N_LOC + n
            cp = (c0 + rel_c) / (H - 1)                     # unit coords
            for k in range(K):
                dxc = cp - mu_x[ng, k]
                qa = -0.5 * A[ng, k] * dxc * dxc
                dxB = -0.5 * Bq[ng, k] * dxc
                scale = dxB
                bias = qa - dxB * mu_y[ng, k]
                dy = yg - mu_y[ng, k]
                qc = -0.5 * C[ng, k] * dy * dy              # [W]
                for j in range(NCO):
                    row = rows0 + 8 * k + j
                    if abs(scale[j]) >= TAU:
                        consts[row, 2 * g] = scale[j]
                        consts[row, 2 * g + 1] = bias[j]
                        ybcq[g, row, :] = yg + qc / scale[j]
                    else:
                        # drop the (tiny) dxB*y term: |error| < tau
                        consts[row, 2 * g] = 1.0
                        consts[row, 2 * g + 1] = bias[j]
                        ybcq[g, row, :] = qc
                statn[g, rows0 + 8 * k:rows0 + 8 * k + 8,
                      NMID * q:NMID * (q + 1)] = M1 * wgt[ng, k]
            # f16 exp-overflow guard: shift this chunk's rows down
            # uniformly (cancels exactly in num/den); qc <= 0 so
            # bias + relu(scale) upper-bounds scale*y + qc + bias
            rows = slice(rows0, rows0 + 32)
            m = consts[rows, 2 * g + 1] + \
                np.maximum(consts[rows, 2 * g], 0.0)
            shift = max(0.0, m.max() - 9.0)
            if shift > 0.0:
                consts[rows, 2 * g + 1] -= shift
        in_maps.append({
            "consts": consts.astype(np.float32),
            "ybcq": ybcq.astype(np.float32),
            "statn": statn.astype(np.float32).astype(np.float16),
            "statd": statd,
            "m2pad": m2pad,
        })
    return in_maps


def _run(height, width, params, trace=False, **trace_kwargs):
    assert int(height) == H and int(width) == W, (height, width)
    if "nc" not in _cache:
        _cache["nc"] = _build_nc()
    nc = _cache["nc"]
    in_maps = _host_precompute(params)
    res = run_bass_kernel_spmd(
        nc, in_maps, core_ids=list(range(N_CORES)), trace=trace, **trace_kwargs
    )
    full = np.empty((48, H, W), dtype=np.float32)
    for core in range(N_CORES):
        o = res.results[core]["out"].astype(np.float32)  # [128, 20, 384] f16
        full[core * N_LOC:(core + 1) * N_LOC] = \
            o[:, :NCHUNK, :].transpose(1, 0, 2).reshape(N_LOC, H, W)
    return full.reshape(16, 3, H, W), res


def kernel(height, width, params):
    out, _ = _run(height, width, params)
    return out
